# revision 1
# baseline (speedup 1.0000x reference)
"""Dual-branch cross-attention block (nn_Attention) on 8 Trainium2 NeuronCores.

Sharding: pure data-parallel over batch B=8 — one batch element per core, no
collectives. Per core:

  * All precision-critical matmuls run as fp8e4 DoubleRow (0.5 cycles/row)
    with exact two-term hi/lo operands: x = fp8(x) + fp8(x - fp8(x)), so the
    product (hi+lo)·(hi+lo) is computed near-exactly (lo·lo dropped where it
    is negligible).  Weights/inputs are split host-side; q/k/fcT on-chip.
  * Scores: k_cat = [k_hi;k_lo] stacked on 128 partitions (DH=64) as the
    stationary operand (slab-broadcast), q duplicated per partition-half as
    the moving operand -> ONE DoubleRow matmul per 128-key tile computes the
    exact two-term q.k product.
  * AV runs transposed (out [qpos, dh+1]) in bf16 with the sumexp ones
    column; normalization becomes a per-partition scalar multiply, then a
    PE transpose puts A back into [feature, seq] layout for the FC.
  * fc is plain bf16; qkv/v/out projections are 3-term DoubleRow.
  * V bias is folded into an effective fc bias host-side; out-proj bias is
    added host-side after the run.

Epilogues are spread across ACT (hi splits, exp), DVE (lo splits, scale),
and GPSIMD/Pool (psum->sbuf copies) so no single engine exceeds the PE.
"""

import numpy as np

import concourse.bass as bass
import concourse.mybir as mybir
import concourse.tile as tile
from concourse import bacc
from concourse.alu_op_type import AluOpType
from concourse.masks import make_identity
from concourse.bass_utils import run_bass_kernel_spmd

F32 = mybir.dt.float32
BF16 = mybir.dt.bfloat16
FP8 = mybir.dt.float8e4
AF = mybir.ActivationFunctionType
DR = mybir.MatmulPerfMode.DoubleRow

B, S, D, H, DH = 8, 512, 768, 12, 64
KT = D // 128           # 6 k-tiles over D
ST = S // 128           # 4 s-tiles
WSC = 32.0              # host weight scale: keeps fp8 lo-terms out of subnormals

QK_WEIGHTS = ["Wq", "Wk", "Wqm", "Wkm"]
V_WEIGHTS = ["Wv", "Wvm"]
O_WEIGHTS = ["Wo", "Wom"]


def build_program():
    nc = bacc.Bacc("TRN2", target_bir_lowering=False, debug=False, num_devices=8)

    # host-prepped inputs: xT two-term fp8 [ki, ko, 2, S]
    xt_d = {n: nc.dram_tensor(n, [128, KT, 2, S], FP8, kind="ExternalInput")
            for n in ("xt_h", "xt_m")}
    # projection weights, two-term fp8 [ki, ko, 2, D]
    wd = {n: nc.dram_tensor(n, [128, KT, 2, D], FP8, kind="ExternalInput")
          for n in QK_WEIGHTS + V_WEIGHTS + O_WEIGHTS}
    # fc weights bf16 [ki, 2D/128, D]
    for n in ("Wfc", "Wfcm"):
        wd[n] = nc.dram_tensor(n, [128, 2 * KT, D], BF16, kind="ExternalInput")
    # biases as column-major [128, KT] f32 (bq..bkm + effective fc biases)
    bd = {n: nc.dram_tensor(n, [128, KT], F32, kind="ExternalInput")
          for n in ("bq", "bk", "bqm", "bkm", "bfce", "bfcme")}
    out_p = nc.dram_tensor("out_p", [S, D], BF16, kind="ExternalOutput")
    out_m = nc.dram_tensor("out_m", [S, D], BF16, kind="ExternalOutput")

    with tile.TileContext(nc) as tc:
        with tc.tile_pool(name="cst", bufs=1) as cst, \
             tc.tile_pool(name="xp", bufs=1) as xp, \
             tc.tile_pool(name="wp", bufs=3) as wp, \
             tc.tile_pool(name="stg", bufs=2) as stg, \
             tc.tile_pool(name="qk", bufs=1) as qkp, \
             tc.tile_pool(name="vp", bufs=1) as vpool, \
             tc.tile_pool(name="et", bufs=6) as etp, \
             tc.tile_pool(name="an", bufs=2) as anp, \
             tc.tile_pool(name="aT", bufs=2) as atp, \
             tc.tile_pool(name="fct", bufs=2) as fctp, \
             tc.tile_pool(name="ost", bufs=4) as ostp, \
             tc.tile_pool(name="bias", bufs=8) as biasp, \
             tc.tile_pool(name="scr", bufs=4) as scr, \
             tc.tile_pool(name="psA", bufs=2, space="PSUM") as psA, \
             tc.tile_pool(name="psS", bufs=2, space="PSUM") as psS, \
             tc.tile_pool(name="psV", bufs=1, space="PSUM") as psV, \
             tc.tile_pool(name="psT", bufs=1, space="PSUM") as psT:

            # ---------------- constants ----------------
            ident_f = cst.tile([128, 128], F32)
            make_identity(nc, ident_f[:])
            ident = cst.tile([128, 128], BF16)
            nc.vector.tensor_copy(out=ident[:], in_=ident_f[:])
            onescol_f = cst.tile([128, 1], F32)
            nc.vector.memset(onescol_f[:], 1.0)
            onescol = cst.tile([128, 1], BF16)
            nc.vector.tensor_copy(out=onescol[:], in_=onescol_f[:])

            def bias_col(n):
                t = biasp.tile([128, KT], F32, tag="bias")
                nc.sync.dma_start(t[:], bd[n][:])
                return t

            def load_w(n, split=False):
                # split: per-chunk DMAs so the startup-critical consumers
                # start on first-chunk arrival; whole-tensor otherwise
                # (fewer per-DMA gaps on the serialized DMA engines)
                if n in ("Wfc", "Wfcm"):
                    t = wp.tile([128, 2 * KT, D], BF16, tag="w")
                    for g in range(4):
                        nc.sync.dma_start(t[:, 3 * g:3 * g + 3],
                                          wd[n][:, 3 * g:3 * g + 3])
                else:
                    t = wp.tile([128, KT, 2, D], FP8, tag="w")
                    for c in range(KT):
                        nc.sync.dma_start(t[:, c], wd[n][:, c])
                return t

            def load_xt(n, split=False):
                t = xp.tile([128, KT, 2, S], FP8, tag=n)
                if split:
                    # ACT-queue issue: runs in parallel with the SP queue's
                    # weight DMAs during startup
                    for c in range(KT):
                        nc.scalar.dma_start(t[:, c], xt_d[n][:, c])
                else:
                    nc.sync.dma_start(t[:], xt_d[n][:])
                return t

            xt2_h = load_xt("xt_h", split=True)

            # --------- 3-term DoubleRow contraction over D (6 chunks) ---------
            # products: (Whi_c, xhi_c) + (Wlo_c, xhi_c) via slab pair
            # (Whi_c, Wlo_c) x broadcast xhi_c; then (Whi_c, xlo_c) packed in
            # pairs of adjacent chunks.
            def dr_contract(get_pt, w2, x2, msl, xsl, n_free):
                for c in range(KT):
                    nc.tensor.matmul(
                        get_pt(), w2[:, c, :, msl],
                        x2[:, c, 0, xsl][:, None, :].broadcast_to([128, 2, n_free]),
                        start=(c == 0), stop=False, perf_mode=DR)
                for j in range(KT // 2):
                    nc.tensor.matmul(
                        get_pt(), w2[:, 2 * j:2 * j + 2, 0, msl],
                        x2[:, 2 * j:2 * j + 2, 1, xsl],
                        start=False, stop=(j == KT // 2 - 1), perf_mode=DR)

            # pre-attention proj psums rotate over psA(2)+psV(1)+psT(1):
            # those banks are idle until the first attention unit, and four
            # buffers fully hide the serial hi->lo epilogue
            _proj_ps = [0]

            def proj_pt():
                i = _proj_ps[0] % 4
                _proj_ps[0] += 1
                pool, tag = [(psA, "proj"), (psA, "proj"),
                             (psV, "av"), (psT, "tr")][i]
                pt = pool.tile([128, S], F32, tag=tag, name=f"projpt{i}")
                return pt

            # --------- q/k projection -> two-term fp8 stage [128, 6, 2, S] ---------
            def proj_qk(w2, bcol, x2):
                st_t = stg.tile([128, KT, 2, S], FP8, tag="stage")
                for m in range(KT):
                    pt = proj_pt()
                    get_pt = (lambda pt=pt: pt[:])
                    dr_contract(get_pt, w2, x2, slice(m * 128, (m + 1) * 128),
                                slice(0, S), S)
                    nc.scalar.activation(st_t[:, m, 0, :], get_pt(), AF.Identity,
                                         bias=bcol[:, m:m + 1])
                    nc.vector.scalar_tensor_tensor(
                        out=st_t[:, m, 1, :], in0=get_pt(), scalar=bcol[:, m:m + 1],
                        in1=st_t[:, m, 0, :], op0=AluOpType.add,
                        op1=AluOpType.subtract)
                return st_t

            # --------- rearrange stage -> q_dup [128, 12, 2, S] (4 sbuf DMAs) ---------
            def rearr_q(st_t, tag):
                q = qkp.tile([128, H, 2, S], FP8, tag=tag)
                for dst_half in range(2):
                    for src_half in range(2):
                        nc.sync.dma_start(
                            q[dst_half * 64:dst_half * 64 + 64, src_half::2, :, :],
                            st_t[src_half * 64:src_half * 64 + 64, :, :, :])
                return q

            # --------- rearrange stage -> k_cat [128, 12, S] (4 sbuf DMAs) ---------
            def rearr_k(st_t, tag):
                k = qkp.tile([128, H, S], FP8, tag=tag)
                for t in range(2):           # hi -> partitions 0:64, lo -> 64:128
                    for src_half in range(2):
                        nc.sync.dma_start(
                            k[t * 64:t * 64 + 64, src_half::2, :],
                            st_t[src_half * 64:src_half * 64 + 64, :, t, :])
                return k

            # --------- v projection (row-major, no bias) -> v_aug bf16 ---------
            def proj_v_chunk(v, w2, x2, st, c, on_act=True, pre_attn=False):
                pt = proj_pt() if pre_attn else psA.tile([128, S], F32,
                                                         tag="proj")
                get_pt = (lambda pt=pt: pt[:, :384])
                xsl = slice(st * 128, (st + 1) * 128)
                msl = slice(c * 384, (c + 1) * 384)
                for cc in range(KT):
                    nc.tensor.matmul(
                        get_pt(),
                        x2[:, cc, :, xsl],
                        w2[:, cc, 0, msl][:, None, :].broadcast_to([128, 2, 384]),
                        start=(cc == 0), stop=False, perf_mode=DR)
                for j in range(KT // 2):
                    nc.tensor.matmul(
                        get_pt(),
                        x2[:, 2 * j:2 * j + 2, 0, xsl],
                        w2[:, 2 * j:2 * j + 2, 1, msl],
                        start=False, stop=(j == KT // 2 - 1), perf_mode=DR)
                src = get_pt().rearrange("p (h d) -> p h d", d=DH)
                if on_act:
                    nc.scalar.activation(
                        v[:, st, c * 6:(c + 1) * 6, 0:DH], src,
                        AF.Copy, scale=1.0 / WSC)
                else:
                    nc.vector.tensor_scalar_mul(
                        out=v[:, st, c * 6:(c + 1) * 6, 0:DH],
                        in0=src, scalar1=1.0 / WSC)

            def proj_v(w2, x2, tag):
                v = vpool.tile([128, ST, H, DH + 1], BF16, tag=tag)
                for st in range(ST):
                    for c in range(2):
                        proj_v_chunk(v, w2, x2, st, c, pre_attn=True)
                    nc.vector.tensor_copy(
                        out=v[:, st, :, DH:DH + 1],
                        in_=onescol[:, None, :].broadcast_to([128, H, 1]))
                return v

            # ---------------- attention unit ----------------
            def attn_unit(h, q, k, v, aT, half):
                ets = []
                for g in range(2):       # two kpos-tile pairs
                    stp = psS.tile([128, 2, S], F32, tag="sc")
                    for i2 in range(2):
                        i = g * 2 + i2
                        nc.tensor.matmul(
                            stp[:, i2, :],
                            k[:, h, i * 128:(i + 1) * 128][:, None, :]
                            .broadcast_to([128, 2, 128]),
                            q[:, h, :, :],
                            start=True, stop=True, perf_mode=DR)
                    et = etp.tile([128, 2, S], BF16, tag="et")
                    nc.scalar.activation(et[:], stp[:], AF.Exp,
                                         scale=1.0 / (8.0 * WSC * WSC))
                    ets.append(et)
                avp = psV.tile([128, ST, DH + 1], F32, tag="av")
                for j in range(ST):
                    for i in range(ST):
                        nc.tensor.matmul(
                            avp[:, j, :],
                            ets[i // 2][:, i % 2, j * 128:(j + 1) * 128],
                            v[:, i, h, :],
                            start=(i == 0), stop=(i == ST - 1))
                recip = scr.tile([128, ST], F32, tag="recip")
                with nc.allow_low_precision(reason="softmax recip"):
                    nc.vector.reciprocal(recip[:], avp[:, :, DH:DH + 1])
                an = anp.tile([128, ST, DH], BF16, tag="an")
                nc.vector.tensor_mul(
                    an[:], avp[:, :, 0:DH],
                    recip[:, :, None].broadcast_to([128, ST, DH]))
                tp = psT.tile([DH, ST, 128], BF16, tag="tr")
                for j in range(ST):
                    nc.tensor.transpose(tp[:, j, :], an[:, j, :], ident[:])
                b0 = (h % 2) * 64
                ko = h // 2
                nc.vector.tensor_copy(
                    out=aT[b0:b0 + 64, half * 6 + ko, :],
                    in_=tp[:].rearrange("p j q -> p (j q)"))

            # ---------------- out projection (3-term DR) ----------------
            def out_chunk(fct, wo2, out_dram, st, c0, cw, on_pool,
                          post_attn=False):
                pt = proj_pt() if post_attn else psA.tile([128, S], F32,
                                                          tag="proj")
                qsl = slice(st * 128, (st + 1) * 128)
                csl = slice(c0, c0 + cw)
                for c in range(KT):
                    nc.tensor.matmul(
                        pt[:, :cw], fct[:, c, :, qsl],
                        wo2[:, c, 0, csl][:, None, :].broadcast_to([128, 2, cw]),
                        start=(c == 0), stop=False, perf_mode=DR)
                for j in range(KT // 2):
                    nc.tensor.matmul(
                        pt[:, :cw], fct[:, 2 * j:2 * j + 2, 0, qsl],
                        wo2[:, 2 * j:2 * j + 2, 1, csl],
                        start=False, stop=(j == KT // 2 - 1), perf_mode=DR)
                ot = ostp.tile([128, 512], BF16, tag="ost")
                if on_pool:
                    nc.scalar.activation(ot[:, :cw], pt[:, :cw], AF.Copy,
                                         scale=1.0 / WSC)
                else:
                    nc.vector.tensor_scalar_mul(out=ot[:, :cw], in0=pt[:, :cw],
                                                scalar1=1.0 / WSC)
                nc.gpsimd.dma_start(out_dram[st * 128:(st + 1) * 128, c0:c0 + cw],
                                    ot[:, :cw])

            # ================= schedule =================
            # SP queue: Wq first (startup-critical), then the early biases;
            # Wk/Wv/xt_m before the rearrange DMAs, which park on the SP
            # queue waiting for the q/k epilogues
            wq = load_w("Wq", split=True)
            bcols = {"bq": bias_col("bq"), "bk": bias_col("bk")}
            wk = load_w("Wk", split=True)
            wv = load_w("Wv", split=True)
            xt2_m = load_xt("xt_m")
            for n in ("bqm", "bkm", "bfce", "bfcme"):
                bcols[n] = bias_col(n)
            q_st = proj_qk(wq, bcols["bq"], xt2_h)
            k_st = proj_qk(wk, bcols["bk"], xt2_h)
            q_dup = rearr_q(q_st, "q")
            k_cat = rearr_k(k_st, "k")
            v = proj_v(wv, xt2_h, "v")

            aTp = atp.tile([128, 2 * KT, S], BF16, tag="aT")
            aTm = atp.tile([128, 2 * KT, S], BF16, tag="aT")

            # a_pp units interleaved with qm/km projections
            wqm = load_w("Wqm")
            bqm = bcols["bqm"]
            wkm = load_w("Wkm")
            bkm = bcols["bkm"]
            qm_st = stg.tile([128, KT, 2, S], FP8, tag="stage")
            km_st = stg.tile([128, KT, 2, S], FP8, tag="stage")

            def proj_mtile(st_t, w2, bcol, x2, m, on_act=True):
                pt = psA.tile([128, S], F32, tag="proj")
                dr_contract(lambda pt=pt: pt[:], w2, x2,
                            slice(m * 128, (m + 1) * 128), slice(0, S), S)
                if on_act:
                    nc.scalar.activation(st_t[:, m, 0, :], pt[:], AF.Identity,
                                         bias=bcol[:, m:m + 1])
                else:
                    nc.vector.tensor_scalar_add(
                        out=st_t[:, m, 0, :], in0=pt[:], scalar1=bcol[:, m:m + 1])
                nc.vector.scalar_tensor_tensor(
                    out=st_t[:, m, 1, :], in0=pt[:], scalar=bcol[:, m:m + 1],
                    in1=st_t[:, m, 0, :], op0=AluOpType.add,
                    op1=AluOpType.subtract)

            # qm projection spread over all 12 a_pp units (DVE-balanced);
            # km + vm projections spread over the a_mp loop (km is not
            # needed until a_mm, so its rearrange fires mid-loop)
            qm_dup = None
            for h in range(H):
                attn_unit(h, q_dup, k_cat, v, aTp, 0)      # a_pp
                if h < KT:
                    proj_mtile(qm_st, wqm, bqm, xt2_m, h, on_act=False)
                if h == KT:
                    qm_dup = rearr_q(qm_st, "qm")

            wvm = load_w("Wvm")
            vm = vpool.tile([128, ST, H, DH + 1], BF16, tag="vm")
            km_cat = None
            for h in range(H):
                attn_unit(h, qm_dup, k_cat, v, aTp, 1)     # a_mp
                if h < KT:
                    proj_mtile(km_st, wkm, bkm, xt2_m, h, on_act=False)
                    if h < ST:
                        nc.vector.tensor_copy(
                            out=vm[:, h, :, DH:DH + 1],
                            in_=onescol[:, None, :].broadcast_to([128, H, 1]))
                if h == KT:
                    km_cat = rearr_k(km_st, "km")
                if h >= 4:
                    proj_v_chunk(vm, wvm, xt2_m, (h - 4) // 2, h % 2,
                                 on_act=False)

            # branch-p fc interleaved with a_mm units
            wfc = load_w("Wfc")
            bfce = bcols["bfce"]
            fct_p = fctp.tile([128, KT, 2, S], FP8, tag="fct")

            def fc_mtile(fct, w, bcol, aT, m, on_act=True, post_attn=False):
                pt = proj_pt() if post_attn else psA.tile([128, S], F32,
                                                          tag="proj")
                for kk in range(2 * KT):
                    nc.tensor.matmul(pt[:], w[:, kk, m * 128:(m + 1) * 128],
                                     aT[:, kk, :],
                                     start=(kk == 0), stop=(kk == 2 * KT - 1))
                if on_act:
                    nc.scalar.activation(fct[:, m, 0, :], pt[:], AF.Identity,
                                         bias=bcol[:, m:m + 1])
                else:
                    nc.vector.tensor_scalar_add(
                        out=fct[:, m, 0, :], in0=pt[:], scalar1=bcol[:, m:m + 1])
                nc.vector.scalar_tensor_tensor(
                    out=fct[:, m, 1, :], in0=pt[:], scalar=bcol[:, m:m + 1],
                    in1=fct[:, m, 0, :], op0=AluOpType.add,
                    op1=AluOpType.subtract)

            # fc-p m-tiles spread over both a_mm and a_pm loops (every other
            # unit) so the PE load stays under the ACT exp period
            for h in range(H):
                attn_unit(h, qm_dup, km_cat, vm, aTm, 0)   # a_mm
                if h % 3 == 2:
                    fc_mtile(fct_p, wfc, bfce, aTp, h // 3, on_act=False)

            # branch-p out-proj interleaved with a_pm units
            wo2 = load_w("Wo")
            oc = [(st, c0, cw) for st in range(ST) for c0, cw in ((0, 512), (512, 256))]
            for h in range(H):
                attn_unit(h, q_dup, km_cat, vm, aTm, 1)    # a_pm
                if h in (1, 3):
                    fc_mtile(fct_p, wfc, bfce, aTp, 4 + h // 2, on_act=False)
                elif h >= 6:
                    st, c0, cw = oc[h - 6]
                    out_chunk(fct_p, wo2, out_p, st, c0, cw, on_pool=False)
            # last out-p chunks fill the gap while aTm's final epilogue
            # lands (keeps the PE ramped into fc-m)
            for i in (6, 7):
                st, c0, cw = oc[i]
                out_chunk(fct_p, wo2, out_p, st, c0, cw, on_pool=False,
                          post_attn=True)

            # branch-m fc + out: psums rotate through all four buffers so
            # the hi/lo epilogues never stall the PE
            wfcm = load_w("Wfcm")
            bfcme = bcols["bfcme"]
            fct_m = fctp.tile([128, KT, 2, S], FP8, tag="fct")
            for m in range(KT):
                fc_mtile(fct_m, wfcm, bfcme, aTm, m, post_attn=True)
            wom2 = load_w("Wom")
            for i, (st, c0, cw) in enumerate(oc):
                out_chunk(fct_m, wom2, out_m, st, c0, cw, on_pool=True,
                          post_attn=True)

    nc.compile()
    return nc


_PROGRAM_CACHE = {}


def _fp8_split(a):
    """Exact-ish two-term fp8 split along a new axis: a ~= hi + lo."""
    import ml_dtypes
    a = np.asarray(a, np.float32)
    hi = a.astype(ml_dtypes.float8_e4m3)
    lo = (a - hi.astype(np.float32)).astype(ml_dtypes.float8_e4m3)
    return hi, lo


def prepare_in_maps(inputs):
    """Full-input dict -> per-core in_maps with host-side dtype prep."""
    import ml_dtypes

    shared = {}

    def w_t2(w):
        # [D(=K), M] -> [ki=128, ko=KT, 2, M] two-term fp8, scaled by WSC
        w = np.asarray(w, np.float32) * WSC
        hi, lo = _fp8_split(w)
        kt = w.shape[0] // 128
        out = np.empty((128, kt, 2, w.shape[1]), dtype=ml_dtypes.float8_e4m3)
        out[:, :, 0, :] = hi.reshape(kt, 128, -1).transpose(1, 0, 2)
        out[:, :, 1, :] = lo.reshape(kt, 128, -1).transpose(1, 0, 2)
        return out

    for n in QK_WEIGHTS + V_WEIGHTS + O_WEIGHTS:
        shared[n] = w_t2(inputs[n])
    for n in ("Wfc", "Wfcm"):
        w = np.asarray(inputs[n], np.float32).astype(ml_dtypes.bfloat16)
        shared[n] = np.ascontiguousarray(
            w.reshape(2 * KT, 128, D).transpose(1, 0, 2))

    def b_col(b):
        return np.ascontiguousarray(
            np.asarray(b, np.float32).reshape(KT, 128).T)

    for n in ("bq", "bk", "bqm", "bkm"):
        shared[n] = b_col(np.asarray(inputs[n], np.float32) * WSC)
    # effective fc bias: bfc + [bv;bv] @ Wfc   (v bias folded through concat)
    bfce = np.asarray(inputs["bfc"], np.float32) + np.concatenate(
        [inputs["bv"], inputs["bv"]]).astype(np.float32) @ np.asarray(
            inputs["Wfc"], np.float32)
    bfcme = np.asarray(inputs["bfcm"], np.float32) + np.concatenate(
        [inputs["bvm"], inputs["bvm"]]).astype(np.float32) @ np.asarray(
            inputs["Wfcm"], np.float32)
    shared["bfce"] = b_col(bfce)
    shared["bfcme"] = b_col(bfcme)

    def xt_t2(x):
        # [S, D] -> transposed two-term fp8 [ki=128, ko=KT, 2, S]
        xt = np.ascontiguousarray(np.asarray(x, np.float32).T)  # [D, S]
        hi, lo = _fp8_split(xt)
        out = np.empty((128, KT, 2, S), dtype=ml_dtypes.float8_e4m3)
        out[:, :, 0, :] = hi.reshape(KT, 128, S).transpose(1, 0, 2)
        out[:, :, 1, :] = lo.reshape(KT, 128, S).transpose(1, 0, 2)
        return out

    hs = np.asarray(inputs["hidden_states"], np.float32)
    ml = np.asarray(inputs["mol"], np.float32)
    return [dict(shared, xt_h=xt_t2(hs[b]), xt_m=xt_t2(ml[b]))
            for b in range(B)]


def kernel(hidden_states, mol, Wq, bq, Wk, bk, Wv, bv, Wqm, bqm, Wkm, bkm,
           Wvm, bvm, Wfc, bfc, Wfcm, bfcm, Wo, bo, Wom, bom):
    if "nc" not in _PROGRAM_CACHE:
        _PROGRAM_CACHE["nc"] = build_program()
    nc = _PROGRAM_CACHE["nc"]
    in_maps = prepare_in_maps(dict(
        hidden_states=hidden_states, mol=mol, Wq=Wq, bq=bq, Wk=Wk, bk=bk,
        Wv=Wv, bv=bv, Wqm=Wqm, bqm=bqm, Wkm=Wkm, bkm=bkm, Wvm=Wvm, bvm=bvm,
        Wfc=Wfc, bfc=bfc, Wfcm=Wfcm, bfcm=bfcm, Wo=Wo, bo=bo, Wom=Wom, bom=bom))

    res = run_bass_kernel_spmd(nc, in_maps, core_ids=list(range(B)))
    bo32 = np.asarray(bo, np.float32)
    bom32 = np.asarray(bom, np.float32)
    attn_prot = np.stack(
        [res.results[b]["out_p"].astype(np.float32) + bo32 for b in range(B)])
    attn_mol = np.stack(
        [res.results[b]["out_m"].astype(np.float32) + bom32 for b in range(B)])
    return attn_prot, attn_mol

